# revision 1
# baseline (speedup 1.0000x reference)
"""Trainium2 Bass kernel for per-edge dot products (GNN DotPredictor).

score[e] = sum_d h[src[e], d] * h[dst[e], d]

h: [100000, 64] f32, src/dst: [1250000] int -> score: [1250000] f32.

Strategy: edge-parallel across 8 NeuronCores; each core gets the full h
table in its HBM plus a shard of 156250 edges. The gather engine of choice
is GPSIMD dma_gather (the production embedding-gather op): it consumes a
wrapped int16 index list and fetches one 256B row per index from HBM at
full DMA-engine parallelism. Since indices are int16 (< 32768) the node
table is viewed as 4 chunks of 32768 rows, and edges are binned host-side
into 16 buckets by the (src_chunk, dst_chunk) pair so every gather in a
bucket uses in-range chunk-local indices. The vector engine multiplies
gathered src/dst rows and does a segmented sum over the 64-wide feature
dim. The host inverts the bucket permutation when unsharding.
"""

import numpy as np

N_NODES = 100000
D = 64
E = 1250000
NCORES = 8
P = 128
CHUNK = 32768           # int16-addressable rows per table chunk
NVAR = 4                # node-table chunks per endpoint
NBUCKET = NVAR * NVAR
E_PC = E // NCORES      # 156250 edges per core

_CACHE = {}


def _build(caps, reps=1):
    """Build + compile the Bass program for bucket capacities `caps`
    (tuple of 16 ints, each a multiple of 128, possibly 0).

    reps > 1 repeats the whole workload (for differential timing)."""
    import concourse.bass as bass  # noqa: F401
    import concourse.mybir as mybir
    import concourse.tile as tile
    from concourse import bacc

    sc = sum(caps) // P        # score columns
    s_tot = sum(caps) // 16    # index columns

    nc = bacc.Bacc(
        "TRN2",
        target_bir_lowering=False,
        debug=False,
        enable_asserts=False,
        num_swdge_queues=4,
    )

    h = nc.dram_tensor("h", [N_NODES, D], mybir.dt.float32, kind="ExternalInput")
    sidx = nc.dram_tensor("sidx", [P, s_tot], mybir.dt.int16, kind="ExternalInput")
    didx = nc.dram_tensor("didx", [P, s_tot], mybir.dt.int16, kind="ExternalInput")
    out = nc.dram_tensor("out", [P, sc], mybir.dt.float32, kind="ExternalOutput")

    with tile.TileContext(nc) as tc:
        with (
            tc.tile_pool(name="idx", bufs=1) as idx_pool,
            tc.tile_pool(name="g", bufs=4) as gpool,
            tc.tile_pool(name="acc", bufs=1) as apool,
        ):
            sidx_t = idx_pool.tile([P, s_tot], mybir.dt.int16, tag="sidx")
            didx_t = idx_pool.tile([P, s_tot], mybir.dt.int16, tag="didx")
            scores = apool.tile([P, sc], mybir.dt.float32, tag="scores")

            nc.sync.dma_start(sidx_t[:], sidx[:])
            nc.sync.dma_start(didx_t[:], didx[:])

            SB = 8192  # max indices per dma_gather (descriptor carveout)
            for _rep in range(reps):
                _emit_body(nc, mybir, caps, h, sidx_t, didx_t, scores, gpool, SB)

            nc.sync.dma_start(out[:], scores[:])

    # Spread gathers across the 4 SWDGE queues for more in-flight DMA.
    # Must follow the SCHEDULED Pool-engine order so each of Tile's 8
    # DMASW sem lanes (assigned round-robin in that order) sees a single
    # queue (8 lanes % 4 queues aligns; ucode locks each sem to a queue).
    cnt = 0
    for blk in nc.m.functions[0].blocks:
        for inst in blk.instructions:
            if isinstance(inst, mybir.InstDMAGatherAnt):
                inst.queue_num = cnt % 4
                cnt += 1
    assert cnt > 0

    nc.compile()
    return nc


def _emit_body(nc, mybir, caps, h, sidx_t, didx_t, scores, gpool, SB):
    col = 0   # score/output column offset
    scol = 0  # index column offset
    if True:
            for b, cap in enumerate(caps):
                if cap == 0:
                    continue
                sv, dv = b // NVAR, b % NVAR
                h_src = h[sv * CHUNK:min((sv + 1) * CHUNK, N_NODES), :]
                h_dst = h[dv * CHUNK:min((dv + 1) * CHUNK, N_NODES), :]
                for off in range(0, cap, SB):
                    n = min(SB, cap - off)
                    bb = n // P     # gathered blocks per partition
                    sb = n // 16    # index columns for this sub-batch
                    gs = gpool.tile([P, SB // P, D], mybir.dt.float32, tag="gs")
                    gd = gpool.tile([P, SB // P, D], mybir.dt.float32, tag="gd")

                    nc.gpsimd.dma_gather(
                        out_ap=gs[:, :bb, :],
                        in_ap=h_src,
                        idxs_ap=sidx_t[:, scol:scol + sb],
                        num_idxs=n,
                        num_idxs_reg=n,
                        elem_size=D,
                        single_packet=False,
                    )
                    nc.gpsimd.dma_gather(
                        out_ap=gd[:, :bb, :],
                        in_ap=h_dst,
                        idxs_ap=didx_t[:, scol:scol + sb],
                        num_idxs=n,
                        num_idxs_reg=n,
                        elem_size=D,
                        single_packet=False,
                    )
                    # in-place product then segmented sum over features
                    gss = gs[:, :bb, :]
                    nc.vector.tensor_tensor(
                        out=gss, in0=gss, in1=gd[:, :bb, :],
                        op=mybir.AluOpType.mult
                    )
                    nc.vector.tensor_reduce(
                        out=scores[:, col:col + bb],
                        in_=gss,
                        axis=mybir.AxisListType.X,
                        op=mybir.AluOpType.add,
                    )
                    col += bb
                    scol += sb


def get_nc(caps, reps=1):
    key = (tuple(caps), reps)
    if key not in _CACHE:
        _CACHE[key] = _build(key[0], reps)
    return _CACHE[key]


def _wrap16(idx16):
    """Linear int16 index list (len % 16 == 0) -> [128, len/16] wrapped tile
    (element i at [i % 16, i // 16], replicated for the 8 Q7 cores)."""
    w = idx16.reshape(-1, 16).T
    return np.tile(w, (8, 1))


def _prepare(src32, dst32):
    """Bin each core's edge shard into 16 (src_chunk, dst_chunk) buckets.

    Returns (caps, in-map fragments per core, scatter positions per core).
    """
    per_core = []
    for i in range(NCORES):
        s = src32[i * E_PC:(i + 1) * E_PC]
        d = dst32[i * E_PC:(i + 1) * E_PC]
        # endpoint variant: node-table chunk
        sv = s >> 15
        dv = d >> 15
        bucket = sv * NVAR + dv
        # secondary sort by src for HBM locality in the src gather stream
        perm = np.lexsort((s, bucket))
        counts = np.bincount(bucket, minlength=NBUCKET)
        per_core.append((s, d, bucket, perm, counts))

    all_counts = np.stack([pc[4] for pc in per_core])
    caps = tuple(int(-(-c // P) * P) for c in all_counts.max(axis=0))
    sc = sum(caps) // P

    col_off = np.concatenate([[0], np.cumsum([c // P for c in caps])])

    frags = []
    for s, d, bucket, perm, counts in per_core:
        ssort = s[perm]
        dsort = d[perm]
        cum = np.concatenate([[0], np.cumsum(counts)])
        sidx = np.zeros(sum(caps), np.int16)
        didx = np.zeros(sum(caps), np.int16)
        # flat HBM position of each sorted edge's score: p*sc + col
        pos = np.empty(E_PC, np.int64)
        off = 0
        for b in range(NBUCKET):
            n = int(counts[b])
            lo, hi = int(cum[b]), int(cum[b + 1])
            sidx[off:off + n] = ssort[lo:hi] & 0x7FFF
            didx[off:off + n] = dsort[lo:hi] & 0x7FFF
            i_local = np.arange(n)
            pos[lo:hi] = (i_local % P) * sc + col_off[b] + i_local // P
            off += caps[b]
        frags.append(
            {
                "sidx": _wrap16(sidx),
                "didx": _wrap16(didx),
                "perm": perm,
                "pos": pos,
            }
        )
    return caps, frags


def run_sharded(h, src, dst, trace=False, **kwargs):
    """Run the SPMD kernel; returns (full_output, BassKernelResults)."""
    from concourse.bass_utils import run_bass_kernel_spmd

    h32 = np.ascontiguousarray(np.asarray(h), dtype=np.float32)
    src32 = np.asarray(src).astype(np.int32)
    dst32 = np.asarray(dst).astype(np.int32)

    caps, frags = _prepare(src32, dst32)
    nc = get_nc(caps)

    in_maps = [
        {"h": h32, "sidx": f["sidx"], "didx": f["didx"]} for f in frags
    ]
    res = run_bass_kernel_spmd(
        nc, in_maps, core_ids=list(range(NCORES)), trace=trace, **kwargs
    )

    full = np.empty(E, np.float32)
    for i, f in enumerate(frags):
        flat = np.asarray(res.results[i]["out"]).reshape(-1)
        shard = np.empty(E_PC, np.float32)
        shard[f["perm"]] = flat[f["pos"]]
        full[i * E_PC:(i + 1) * E_PC] = shard
    return full, res


def kernel(h, src, dst):
    full, _ = run_sharded(h, src, dst, trace=False)
    return full



# revision 2
# speedup vs baseline: 1.6053x; 1.6053x over previous
"""Trainium2 Bass kernel for per-edge dot products (GNN DotPredictor).

score[e] = sum_d h[src[e], d] * h[dst[e], d]

h: [100000, 64] f32, src/dst: [1250000] int -> score: [1250000] f32.

Strategy: edge-parallel across 8 NeuronCores; each core gets the full h
table in its HBM plus a shard of 156250 edges. The gather engine of choice
is GPSIMD dma_gather (the production embedding-gather op): it consumes a
wrapped int16 index list and fetches one 256B row per index from HBM at
full DMA-engine parallelism. Since indices are int16 (< 32768) the node
table is viewed as 4 chunks of 32768 rows, and edges are binned host-side
into 16 buckets by the (src_chunk, dst_chunk) pair so every gather in a
bucket uses in-range chunk-local indices. The vector engine multiplies
gathered src/dst rows and does a segmented sum over the 64-wide feature
dim. The host inverts the bucket permutation when unsharding.
"""

import numpy as np

N_NODES = 100000
D = 64
E = 1250000
NCORES = 8
P = 128
CHUNK = 32768           # int16-addressable rows per table chunk
NVAR = 4                # node-table chunks per endpoint
NBUCKET = NVAR * NVAR
E_PC = E // NCORES      # 156250 edges per core

_CACHE = {}


def _build(caps, reps=1):
    """Build + compile the Bass program for bucket capacities `caps`
    (tuple of 16 ints, each a multiple of 128, possibly 0).

    reps > 1 repeats the whole workload (for differential timing)."""
    import concourse.bass as bass  # noqa: F401
    import concourse.mybir as mybir
    import concourse.tile as tile
    from concourse import bacc

    sc = sum(caps) // P        # score columns
    s_tot = sum(caps) // 16    # index columns

    nc = bacc.Bacc(
        "TRN2",
        target_bir_lowering=False,
        debug=False,
        enable_asserts=False,
        num_swdge_queues=4,
    )

    h = nc.dram_tensor("h", [N_NODES, D], mybir.dt.float32, kind="ExternalInput")
    sidx = nc.dram_tensor("sidx", [P, s_tot], mybir.dt.int16, kind="ExternalInput")
    didx = nc.dram_tensor("didx", [P, s_tot], mybir.dt.int16, kind="ExternalInput")
    out = nc.dram_tensor("out", [P, sc], mybir.dt.float32, kind="ExternalOutput")

    with tile.TileContext(nc) as tc:
        with (
            tc.tile_pool(name="idx", bufs=1) as idx_pool,
            tc.tile_pool(name="g", bufs=4) as gpool,
            tc.tile_pool(name="acc", bufs=1) as apool,
        ):
            sidx_t = idx_pool.tile([P, s_tot], mybir.dt.int16, tag="sidx")
            didx_t = idx_pool.tile([P, s_tot], mybir.dt.int16, tag="didx")
            scores = apool.tile([P, sc], mybir.dt.float32, tag="scores")

            nc.sync.dma_start(sidx_t[:], sidx[:])
            nc.sync.dma_start(didx_t[:], didx[:])

            # Indices per dma_gather call. Smaller batches keep more
            # gathers in flight across the 4 SWDGE queues (measured ~1.7x
            # faster than 8192 on HW; the gather is descriptor-rate-bound,
            # not byte-bound).
            SB = 3072
            for _rep in range(reps):
                _emit_body(nc, mybir, caps, h, sidx_t, didx_t, scores, gpool, SB)

            nc.sync.dma_start(out[:], scores[:])

    # Spread gathers across the 4 SWDGE queues for more in-flight DMA.
    # Must follow the SCHEDULED Pool-engine order so each of Tile's 8
    # DMASW sem lanes (assigned round-robin in that order) sees a single
    # queue (8 lanes % 4 queues aligns; ucode locks each sem to a queue).
    cnt = 0
    for blk in nc.m.functions[0].blocks:
        for inst in blk.instructions:
            if isinstance(inst, mybir.InstDMAGatherAnt):
                inst.queue_num = cnt % 4
                cnt += 1
    assert cnt > 0

    nc.compile()
    return nc


def _emit_body(nc, mybir, caps, h, sidx_t, didx_t, scores, gpool, SB):
    col = 0   # score/output column offset
    scol = 0  # index column offset
    if True:
            for b, cap in enumerate(caps):
                if cap == 0:
                    continue
                sv, dv = b // NVAR, b % NVAR
                h_src = h[sv * CHUNK:min((sv + 1) * CHUNK, N_NODES), :]
                h_dst = h[dv * CHUNK:min((dv + 1) * CHUNK, N_NODES), :]
                for off in range(0, cap, SB):
                    n = min(SB, cap - off)
                    bb = n // P     # gathered blocks per partition
                    sb = n // 16    # index columns for this sub-batch
                    gs = gpool.tile([P, SB // P, D], mybir.dt.float32, tag="gs")
                    gd = gpool.tile([P, SB // P, D], mybir.dt.float32, tag="gd")

                    nc.gpsimd.dma_gather(
                        out_ap=gs[:, :bb, :],
                        in_ap=h_src,
                        idxs_ap=sidx_t[:, scol:scol + sb],
                        num_idxs=n,
                        num_idxs_reg=n,
                        elem_size=D,
                        single_packet=False,
                    )
                    nc.gpsimd.dma_gather(
                        out_ap=gd[:, :bb, :],
                        in_ap=h_dst,
                        idxs_ap=didx_t[:, scol:scol + sb],
                        num_idxs=n,
                        num_idxs_reg=n,
                        elem_size=D,
                        single_packet=False,
                    )
                    # in-place product then segmented sum over features
                    gss = gs[:, :bb, :]
                    nc.vector.tensor_tensor(
                        out=gss, in0=gss, in1=gd[:, :bb, :],
                        op=mybir.AluOpType.mult
                    )
                    nc.vector.tensor_reduce(
                        out=scores[:, col:col + bb],
                        in_=gss,
                        axis=mybir.AxisListType.X,
                        op=mybir.AluOpType.add,
                    )
                    col += bb
                    scol += sb


def get_nc(caps, reps=1):
    key = (tuple(caps), reps)
    if key not in _CACHE:
        _CACHE[key] = _build(key[0], reps)
    return _CACHE[key]


def _wrap16(idx16):
    """Linear int16 index list (len % 16 == 0) -> [128, len/16] wrapped tile
    (element i at [i % 16, i // 16], replicated for the 8 Q7 cores)."""
    w = idx16.reshape(-1, 16).T
    return np.tile(w, (8, 1))


def _prepare(src32, dst32):
    """Bin each core's edge shard into 16 (src_chunk, dst_chunk) buckets.

    Returns (caps, in-map fragments per core, scatter positions per core).
    """
    per_core = []
    for i in range(NCORES):
        s = src32[i * E_PC:(i + 1) * E_PC]
        d = dst32[i * E_PC:(i + 1) * E_PC]
        # endpoint variant: node-table chunk
        sv = s >> 15
        dv = d >> 15
        bucket = sv * NVAR + dv
        # secondary sort by src for HBM locality in the src gather stream
        perm = np.lexsort((s, bucket))
        counts = np.bincount(bucket, minlength=NBUCKET)
        per_core.append((s, d, bucket, perm, counts))

    all_counts = np.stack([pc[4] for pc in per_core])
    caps = tuple(int(-(-c // P) * P) for c in all_counts.max(axis=0))
    sc = sum(caps) // P

    col_off = np.concatenate([[0], np.cumsum([c // P for c in caps])])

    frags = []
    for s, d, bucket, perm, counts in per_core:
        ssort = s[perm]
        dsort = d[perm]
        cum = np.concatenate([[0], np.cumsum(counts)])
        sidx = np.zeros(sum(caps), np.int16)
        didx = np.zeros(sum(caps), np.int16)
        # flat HBM position of each sorted edge's score: p*sc + col
        pos = np.empty(E_PC, np.int64)
        off = 0
        for b in range(NBUCKET):
            n = int(counts[b])
            lo, hi = int(cum[b]), int(cum[b + 1])
            sidx[off:off + n] = ssort[lo:hi] & 0x7FFF
            didx[off:off + n] = dsort[lo:hi] & 0x7FFF
            i_local = np.arange(n)
            pos[lo:hi] = (i_local % P) * sc + col_off[b] + i_local // P
            off += caps[b]
        frags.append(
            {
                "sidx": _wrap16(sidx),
                "didx": _wrap16(didx),
                "perm": perm,
                "pos": pos,
            }
        )
    return caps, frags


def run_sharded(h, src, dst, trace=False, **kwargs):
    """Run the SPMD kernel; returns (full_output, BassKernelResults)."""
    from concourse.bass_utils import run_bass_kernel_spmd

    h32 = np.ascontiguousarray(np.asarray(h), dtype=np.float32)
    src32 = np.asarray(src).astype(np.int32)
    dst32 = np.asarray(dst).astype(np.int32)

    caps, frags = _prepare(src32, dst32)
    nc = get_nc(caps)

    in_maps = [
        {"h": h32, "sidx": f["sidx"], "didx": f["didx"]} for f in frags
    ]
    res = run_bass_kernel_spmd(
        nc, in_maps, core_ids=list(range(NCORES)), trace=trace, **kwargs
    )

    full = np.empty(E, np.float32)
    for i, f in enumerate(frags):
        flat = np.asarray(res.results[i]["out"]).reshape(-1)
        shard = np.empty(E_PC, np.float32)
        shard[f["perm"]] = flat[f["pos"]]
        full[i * E_PC:(i + 1) * E_PC] = shard
    return full, res


def kernel(h, src, dst):
    full, _ = run_sharded(h, src, dst, trace=False)
    return full

